# revision 15
# baseline (speedup 1.0000x reference)
"""2-layer GAT (PyG GATConv semantics) on 8 Trainium2 NeuronCores via Bass/Tile.

v8 strategy — dst-node partitioning, host-staged layer-1 edge rows, and a
logit-space layer-2 table:
 - The DMA-gather path costs ~10ns per DESCRIPTOR (size-independent 256B..
   1KB, DGE-bound), so per-edge gathers are the wall.  Layer 1 avoids them
   entirely: the host stages x[src] for every edge slot (pure data layout,
   no weights) in transposed form ("xeT"), and one PE matmul per 128-slot
   column against rhs [W1 | u1 | v1] produces the per-edge xw rows AND the
   attention dots a1s/a1d in one shot (u1/v1 fold att_src1/att_dst1 into
   W1).  Aggregation weights xw rows per head (multi-head forbids
   aggregating raw x), then h = relu(agg/den + b1).
 - Per dst tile: transpose(h) -> @[W2 | u2 | v2] yields the layer-2 logit
   rows l = h@W2 plus a2s = h.u2, a2d = h.v2 (aggregating layer 2 in logit
   space is valid: single head).  T2 rows are 256B: [l bf16 x8 | a2s f32].
 - Layer 2 needs one dma_gather per edge (h is device-computed); gathers
   alternate across 4 SWDGE queues (2-queue parallelism ~1.9x, saturated
   beyond), and phase-2 math is over 8 classes, not 128 features.
 - Uniform-width chunks: tiles are degree-bucketed; every tile in a chunk
   is padded to the chunk max degree so all per-tile vector ops fuse into
   ONE chunked instruction (segment reduces become strided 4D views).
 - The phase-2 gather ring is a separate SBUF ring from the phase-1 xeT
   ring so consecutive reps overlap (phase-1 compute hides under phase-2
   gathers); DVE-only temporaries are single-buffered to pay for it.
 - Per-node attention dst terms (a1d via phase 0, a2d via phase 1) live in
   SBUF; phase 0 is 49 tiny matmuls; the T2loc AllGather is ~40us.
"""
import sys

sys.path.insert(0, "/opt/trn_rl_repo")

import numpy as np

import concourse.bass as bass
import concourse.bacc as bacc
import concourse.mybir as mybir
import concourse.tile as tile
from concourse.tile_rust import add_dep_helper
from concourse.bass_utils import run_bass_kernel_spmd

P = 128
NCORES = 8
F_IN = 128
HEADS = 4
HID = 32
NCLS = 8
NEG_SLOPE = 0.2
EPS = 1e-20
FW = HEADS * HID              # 128 : feature width (both layers)
RW2 = 128                     # T2 row width in bf16 elems (256B pitch)
WIN = 65536                   # table window rows (int16-addressable, signed)
SHARD = 8192                  # window rows per core shard (aligned)
DUMMY_POS = WIN - 1           # trailing dummy gather target (gap row)
L2S = 8                       # bf16 col of a2s f32 in T2 rows (cols 8:10)

f32 = mybir.dt.float32
bf16 = mybir.dt.bfloat16
i16 = mybir.dt.int16

CHUNK_COLS = 48               # slot-columns per chunk (uniform tile width)


def _to_bf16(a):
    import ml_dtypes
    return np.asarray(a, np.float32).astype(ml_dtypes.bfloat16)


# ----------------------------------------------------------------------------
# host-side graph preprocessing
# ----------------------------------------------------------------------------

def preprocess(N, edge_index, n_tiles_per_core):
    """Uniform-width slot layout + per-core edge staging / gather indices."""
    NT = n_tiles_per_core
    slots_pc = NT * P
    total_slots = slots_pc * NCORES
    npad_pc = (total_slots - N) // NCORES
    assert npad_pc * NCORES == total_slots - N and npad_pc >= 1
    assert slots_pc <= SHARD and total_slots <= WIN

    src = np.concatenate([edge_index[0], np.arange(N, dtype=np.int64)]).astype(np.int64)
    dst = np.concatenate([edge_index[1], np.arange(N, dtype=np.int64)]).astype(np.int64)

    deg = np.bincount(dst, minlength=N)  # in-degree incl self loop
    order = np.argsort(deg, kind="stable")  # ascending

    cap0 = (P - npad_pc) * NCORES
    r = np.arange(N)
    core = np.empty(N, np.int64)
    t_of = np.empty(N, np.int64)
    d_of = np.empty(N, np.int64)
    m0 = r < cap0
    core[m0] = r[m0] % NCORES
    t_of[m0] = 0
    d_of[m0] = npad_pc + r[m0] // NCORES
    r2 = r[~m0] - cap0
    core[~m0] = r2 % NCORES
    t_of[~m0] = 1 + r2 // (P * NCORES)
    d_of[~m0] = (r2 % (P * NCORES)) // NCORES
    assert t_of.max() < NT

    # window position of each node's table row (for the layer-2 gather)
    pos_sorted = core * SHARD + t_of * P + d_of
    pos_of = np.empty(N, np.int64)
    pos_of[order] = pos_sorted

    W_prog = np.zeros(NT, np.int64)
    np.maximum.at(W_prog, t_of, deg[order])
    W_prog = np.maximum(W_prog, 1)
    assert W_prog.max() <= CHUNK_COLS

    # uniform-width chunks: (a, b, Wc); W_prog is nondecreasing
    chunks = []
    t0 = 0
    while t0 < NT:
        b = t0 + 1
        while b < NT and (b + 1 - t0) * W_prog[b] <= CHUNK_COLS:
            b += 1
        chunks.append((t0, b, int(W_prog[b - 1])))
        t0 = b
    K = len(chunks)
    C_k = [0]
    for (a, b, Wc) in chunks:
        C_k.append(C_k[-1] + (b - a) * Wc)
    SWu = C_k[-1]
    base_col = np.zeros(NT, np.int64)
    for k, (a, b, Wc) in enumerate(chunks):
        for t in range(a, b):
            base_col[t] = C_k[k] + (t - a) * Wc

    lslot_sorted = t_of * P + d_of
    lslot = np.empty(N, np.int64)
    lslot[order] = lslot_sorted
    core_of = np.empty(N, np.int64)
    core_of[order] = core

    dkey = core_of[dst] * slots_pc + lslot[dst]
    is_self = (src == dst)
    eorder = np.argsort(dkey * 2 + (~is_self).astype(np.int64), kind="stable")
    dk_sorted = dkey[eorder]
    src_sorted = src[eorder]
    starts = np.searchsorted(dk_sorted, np.arange(total_slots))
    k_idx = np.arange(len(dk_sorted)) - starts[dk_sorted]

    e_core = dk_sorted // slots_pc
    e_rem = dk_sorted % slots_pc
    e_t = e_rem // P
    e_d = e_rem % P
    e_col = base_col[e_t] + k_idx
    assert (k_idx < W_prog[e_t]).all()

    # per-edge-slot source node id (pad slots -> node N = zero row)
    xe_src = np.full((NCORES, P, SWu), N, np.int64)
    xe_src[e_core, e_d, e_col] = src_sorted
    mask_all = np.zeros((NCORES, P, SWu), np.float32)
    mask_all[e_core, e_d, e_col] = 1.0

    # layer-2 gather positions, staged with one trailing dummy col per chunk
    posR = np.zeros((NCORES, P, SWu), np.int64)  # pad slots -> position 0
    posR[e_core, e_d, e_col] = pos_of[src_sorted]
    posA = np.full((NCORES, P, SWu + K), DUMMY_POS, np.int64)
    for k, (a, b, Wc) in enumerate(chunks):
        cc = (b - a) * Wc
        posA[:, :, C_k[k] + k:C_k[k] + k + cc] = posR[:, :, C_k[k]:C_k[k] + cc]

    idx16 = ((posA ^ 0x8000) & 0xFFFF).astype(np.uint16)
    idxA = np.zeros((NCORES, P, (SWu + K) * 8), np.int16)
    for c in range(NCORES):
        flat = idx16[c].T.ravel()
        w16 = flat.reshape(-1, 16).T
        idxA[c] = np.tile(w16, (8, 1)).view(np.int16)

    node_of_local = np.full((NCORES, slots_pc), -1, np.int64)
    node_of_local[core_of, lslot] = np.arange(N)

    return dict(
        NT=NT, SWu=SWu, chunks=chunks, C_k=C_k, npad_pc=int(npad_pc),
        xe_src=xe_src, idxA=idxA, mask_all=mask_all, pos_of=pos_of,
        node_of_local=node_of_local, slots_pc=slots_pc,
    )


# ----------------------------------------------------------------------------
# device program
# ----------------------------------------------------------------------------

def build_program(NT, SWu, chunks, C_k, n_reps=1):
    NSH = NT * P
    K = len(chunks)
    TPC = max(b - a for a, b, _ in chunks)
    CC = CHUNK_COLS
    nc = bacc.Bacc("TRN2", target_bir_lowering=False, debug=False,
                   num_devices=NCORES, num_swdge_queues=4)

    tens = {}
    tens["xT"] = nc.dram_tensor("xT", [P, NSH], bf16, kind="ExternalInput")
    tens["xeT"] = nc.dram_tensor("xeT", [P, SWu * FW], bf16,
                                 kind="ExternalInput")
    tens["idxA"] = nc.dram_tensor("idxA", [P, (SWu + K) * 8], i16,
                                  kind="ExternalInput")
    tens["maskin"] = nc.dram_tensor("maskin", [P, SWu], bf16,
                                    kind="ExternalInput")
    tens["u1v1"] = nc.dram_tensor("u1v1", [F_IN, 2 * HEADS], bf16,
                                  kind="ExternalInput")
    tens["W1f"] = nc.dram_tensor("W1f", [F_IN, FW + 2 * HEADS], bf16,
                                 kind="ExternalInput")
    tens["W2e"] = nc.dram_tensor("W2e", [FW, NCLS + 2], bf16,
                                 kind="ExternalInput")
    tens["b1b"] = nc.dram_tensor("b1b", [P, FW], f32, kind="ExternalInput")
    tens["b2b"] = nc.dram_tensor("b2b", [P, NCLS], f32, kind="ExternalInput")
    tens["ident"] = nc.dram_tensor("ident", [P, P], bf16,
                                   kind="ExternalInput")
    t_out = nc.dram_tensor("OUT", [NSH, NCLS], f32, kind="ExternalOutput")

    AluOp = mybir.AluOpType
    Act = mybir.ActivationFunctionType
    AxX = mybir.AxisListType.X

    with tile.TileContext(nc) as tc:
        from contextlib import ExitStack
        es = ExitStack()
        cp = es.enter_context(tc.tile_pool(name="const", bufs=1))
        p0 = es.enter_context(tc.tile_pool(name="p0", bufs=3))
        pg = es.enter_context(tc.tile_pool(name="pg", bufs=2))
        pm = es.enter_context(tc.tile_pool(name="pm", bufs=2))
        pm1 = es.enter_context(tc.tile_pool(name="pm1", bufs=1))
        psp = es.enter_context(tc.tile_pool(name="ps", bufs=1, space="PSUM"))
        psq = es.enter_context(tc.tile_pool(name="psq", bufs=2, space="PSUM"))
        dram = es.enter_context(tc.tile_pool(name="dram", bufs=1, space="DRAM"))

        u1v1_s = cp.tile([F_IN, 2 * HEADS], bf16)
        W1f_s = cp.tile([F_IN, FW + 2 * HEADS], bf16)
        W2e_s = cp.tile([FW, NCLS + 2], bf16)
        b1_s = cp.tile([P, FW], f32)
        b2_s = cp.tile([P, NCLS], f32)
        id_s = cp.tile([P, P], bf16)
        mask_s = cp.tile([P, SWu], bf16)
        idx_s = cp.tile([P, (SWu + K) * 8], i16)
        for sb_t, name in [(u1v1_s, "u1v1"), (W1f_s, "W1f"), (W2e_s, "W2e"),
                           (b1_s, "b1b"), (b2_s, "b2b"), (id_s, "ident"),
                           (mask_s, "maskin"), (idx_s, "idxA")]:
            nc.sync.dma_start(out=sb_t[:], in_=tens[name][:])
        adT = cp.tile([P, NT, 2 * HEADS], f32)    # [a1s|a1d] per own node
        adT2 = cp.tile([P, NT], f32)              # a2d per own node

        T2loc = dram.tile([SHARD, RW2], bf16)
        T2r = [dram.tile([WIN, RW2], bf16, addr_space="Shared",
                         name=f"T2r{r}") for r in range(n_reps)]

        for rep in range(n_reps):
            # ---- phase 0: adT[:, t, :] = [x.u1 | x.v1] for own nodes
            BLK = 8
            for tb in range(0, NT, BLK):
                nb = min(BLK, NT - tb)
                xTb = p0.tile([P, BLK * P], bf16, tag="xTblk")
                nc.sync.dma_start(out=xTb[:, 0:nb * P],
                                  in_=tens["xT"][:, tb * P:(tb + nb) * P])
                ps0 = psp.tile([P, BLK, 2 * HEADS], f32, space="PSUM",
                               tag="ps0")
                for q in range(nb):
                    nc.tensor.matmul(out=ps0[:, q, :],
                                     lhsT=xTb[:, q * P:(q + 1) * P],
                                     rhs=u1v1_s[:], start=True, stop=True)
                nc.scalar.copy(adT[:, tb:tb + nb, :], ps0[:, 0:nb, :])

            # ---- phase 1
            for k in range(K):
                a, b, Wc = chunks[k]
                ntl = b - a
                cc = ntl * Wc
                c0 = C_k[k]
                xeTc = pg.tile([P, CC + 1, FW], bf16, tag="xeT")
                nc.sync.dma_start(
                    out=xeTc[:, 0:cc, :].rearrange("p c f -> p (c f)"),
                    in_=tens["xeT"][:, c0 * FW:(c0 + cc) * FW])
                # per-edge xw rows + attention dots: one matmul per
                # 128-slot column, rhs = [W1 | u1 | v1]
                xw_sb = pm.tile([P, CC, FW + 2 * HEADS], bf16, tag="xw")
                WGB = 3
                for g0 in range(0, cc, WGB):
                    ng = min(WGB, cc - g0)
                    psW = psq.tile([P, WGB, FW + 2 * HEADS], f32,
                                   space="PSUM", tag="psW")
                    for j in range(ng):
                        nc.tensor.matmul(out=psW[:, j, :],
                                         lhsT=xeTc[:, g0 + j, :],
                                         rhs=W1f_s[:], start=True, stop=True)
                    nc.scalar.copy(xw_sb[:, g0:g0 + ng, :], psW[:, 0:ng, :])
                e1 = pm.tile([P, CC, HEADS], f32, tag="e1")
                nc.vector.tensor_tensor(
                    out=e1[:, 0:cc, :].rearrange("p (t w) h -> p t w h",
                                                 w=Wc),
                    in0=xw_sb[:, 0:cc, FW:FW + HEADS].rearrange(
                        "p (t w) h -> p t w h", w=Wc),
                    in1=adT[:, a:b, HEADS:2 * HEADS]
                        .rearrange("p t (w h) -> p t w h", w=1)
                        .to_broadcast([P, ntl, Wc, HEADS]),
                    op=AluOp.add)
                f1 = pm.tile([P, CC, HEADS], f32, tag="f1")
                nc.scalar.activation(f1[:, 0:cc, :], e1[:, 0:cc, :],
                                     Act.Prelu, alpha=NEG_SLOPE)
                ee1 = pm.tile([P, CC, HEADS], bf16, tag="ee1")
                nc.scalar.activation(ee1[:, 0:cc, :], f1[:, 0:cc, :], Act.Exp)
                eem = pm.tile([P, CC, HEADS], bf16, tag="eem")
                nc.vector.tensor_tensor(
                    out=eem[:, 0:cc, :], in0=ee1[:, 0:cc, :],
                    in1=mask_s[:, c0:c0 + cc]
                        .rearrange("p (w h) -> p w h", h=1)
                        .to_broadcast([P, cc, HEADS]),
                    op=AluOp.mult)
                den = pm.tile([P, TPC * HEADS], f32, tag="den")
                nc.vector.tensor_reduce(
                    out=den[:, 0:ntl * HEADS].rearrange(
                        "p (t h) -> p t h", h=HEADS),
                    in_=eem[:, 0:cc, :].rearrange("p (t w) h -> p t h w",
                                                  w=Wc),
                    axis=AxX, op=AluOp.add)
                nc.vector.tensor_scalar_add(den[:, 0:ntl * HEADS],
                                            den[:, 0:ntl * HEADS], EPS)
                rec = pm.tile([P, TPC * HEADS], f32, tag="rec")
                nc.vector.reciprocal(rec[:, 0:ntl * HEADS],
                                     den[:, 0:ntl * HEADS])
                tm = pm1.tile([P, CC, FW], bf16, tag="tm")
                nc.vector.tensor_tensor(
                    out=tm[:, 0:cc, :].rearrange("p c (h k) -> p c h k",
                                                 h=HEADS),
                    in0=xw_sb[:, 0:cc, 0:FW].rearrange(
                        "p c (h k) -> p c h k", h=HEADS),
                    in1=eem[:, 0:cc, :].rearrange("p c (h k) -> p c h k",
                                                  k=1)
                        .to_broadcast([P, cc, HEADS, HID]),
                    op=AluOp.mult)
                agg = pm1.tile([P, TPC * FW], f32, tag="agg")
                nc.vector.tensor_reduce(
                    out=agg[:, 0:ntl * FW].rearrange(
                        "p (t f) -> p t f", f=FW),
                    in_=tm[:, 0:cc, :].rearrange("p (t w) f -> p t f w",
                                                 w=Wc),
                    axis=AxX, op=AluOp.add)
                agn = pm1.tile([P, TPC, FW], f32, tag="agn")
                nc.vector.tensor_tensor(
                    out=agn[:, 0:ntl, :].rearrange("p t (h k) -> p t h k",
                                                   h=HEADS),
                    in0=agg[:, 0:ntl * FW].rearrange(
                        "p (t h k) -> p t h k", t=ntl, h=HEADS),
                    in1=rec[:, 0:ntl * HEADS]
                        .rearrange("p (t h k) -> p t h k", t=ntl, k=1)
                        .to_broadcast([P, ntl, HEADS, HID]),
                    op=AluOp.mult)
                # h = relu(agn + b1); then per tile @[W2|u2|v2]
                nc.vector.tensor_tensor(
                    out=agn[:, 0:ntl, :], in0=agn[:, 0:ntl, :],
                    in1=b1_s[:].rearrange("p (t f) -> p t f", t=1)
                        .to_broadcast([P, ntl, FW]),
                    op=AluOp.add)
                hfb = pm.tile([P, TPC, FW], bf16, tag="hfb")
                nc.scalar.activation(hfb[:, 0:ntl, :], agn[:, 0:ntl, :],
                                     Act.Relu)
                ps2 = psq.tile([P, TPC, NCLS + 2], f32, space="PSUM",
                               tag="ps2")
                for i in range(ntl):
                    psT2 = psp.tile([P, P], bf16, space="PSUM", tag="psT")
                    nc.tensor.transpose(out=psT2[:], in_=hfb[:, i, :],
                                        identity=id_s[:])
                    hT = pm.tile([P, P], bf16, tag="hT")
                    nc.scalar.copy(hT[:], psT2[:])
                    nc.tensor.matmul(out=ps2[:, i, :], lhsT=hT[:],
                                     rhs=W2e_s[:], start=True, stop=True)
                hr2 = pm.tile([P, TPC, RW2], bf16, tag="hr2")
                nc.scalar.copy(hr2[:, 0:ntl, 0:NCLS], ps2[:, 0:ntl, 0:NCLS])
                nc.scalar.copy(hr2[:, 0:ntl, L2S:L2S + 2].bitcast(f32),
                               ps2[:, 0:ntl, NCLS:NCLS + 1])
                nc.scalar.copy(adT2[:, a:b], ps2[:, 0:ntl, NCLS + 1])
                nc.sync.dma_start(
                    out=T2loc[a * P:(a + ntl) * P, :].rearrange(
                        "(q p) f -> p q f", q=ntl),
                    in_=hr2[:, 0:ntl, :])

            T2 = T2r[rep]
            ci = nc.gpsimd.collective_compute(
                "AllGather", AluOp.bypass,
                replica_groups=[list(range(NCORES))],
                ins=[T2loc[:].opt()], outs=[T2[:].opt()])

            # ---- phase 2
            for k in range(K):
                a, b, Wc = chunks[k]
                ntl = b - a
                cc = ntl * Wc
                c0 = C_k[k]
                sc = c0 + k
                g2 = pg.tile([P, CC + 1, FW], bf16, tag="g2")
                gi = nc.gpsimd.dma_gather(
                    g2[:, 0:cc + 1, :], T2[SHARD * 4:WIN, :],
                    idx_s[:, sc * 8:(sc + cc + 1) * 8],
                    (cc + 1) * P, (cc + 1) * P, RW2, single_packet=False,
                    queue_num=k % 4)
                add_dep_helper(gi.ins, ci.ins, True,
                               "gather reads below tracked base via neg idx")
                e2 = pm.tile([P, CC, 1], f32, tag="e2")
                nc.vector.tensor_tensor(
                    out=e2[:, 0:cc, :].rearrange("p (t w) c -> p t w c",
                                                 w=Wc),
                    in0=g2[:, 0:cc, L2S:L2S + 2].bitcast(f32)
                        .rearrange("p (t w) c -> p t w c", w=Wc),
                    in1=adT2[:, a:b].rearrange("p (t w) -> p t w", w=1)
                        .rearrange("p t (w c) -> p t w c", c=1)
                        .to_broadcast([P, ntl, Wc, 1]),
                    op=AluOp.add)
                f2 = pm.tile([P, CC, 1], f32, tag="f2")
                nc.scalar.activation(f2[:, 0:cc, :], e2[:, 0:cc, :],
                                     Act.Prelu, alpha=NEG_SLOPE)
                ee2 = pm.tile([P, CC], bf16, tag="ee2")
                nc.scalar.activation(ee2[:, 0:cc], f2[:, 0:cc, 0], Act.Exp)
                eem2 = pm.tile([P, CC], bf16, tag="eem2")
                nc.vector.tensor_tensor(out=eem2[:, 0:cc], in0=ee2[:, 0:cc],
                                        in1=mask_s[:, c0:c0 + cc],
                                        op=AluOp.mult)
                den2 = pm.tile([P, TPC], f32, tag="den2")
                nc.vector.tensor_reduce(
                    out=den2[:, 0:ntl],
                    in_=eem2[:, 0:cc].rearrange("p (t w) -> p t w", w=Wc),
                    axis=AxX, op=AluOp.add)
                nc.vector.tensor_scalar_add(den2[:, 0:ntl], den2[:, 0:ntl],
                                            EPS)
                rec2 = pm.tile([P, TPC], f32, tag="rec2")
                nc.vector.reciprocal(rec2[:, 0:ntl], den2[:, 0:ntl])
                tm2 = pm.tile([P, CC, NCLS], bf16, tag="tm2")
                nc.vector.tensor_tensor(
                    out=tm2[:, 0:cc, :], in0=g2[:, 0:cc, 0:NCLS],
                    in1=eem2[:, 0:cc].rearrange("p (c k) -> p c k", k=1)
                        .to_broadcast([P, cc, NCLS]),
                    op=AluOp.mult)
                lg = pm.tile([P, TPC * NCLS], f32, tag="lg")
                nc.vector.tensor_reduce(
                    out=lg[:, 0:ntl * NCLS].rearrange(
                        "p (t c) -> p t c", c=NCLS),
                    in_=tm2[:, 0:cc, :].rearrange("p (t w) c -> p t c w",
                                                  w=Wc),
                    axis=AxX, op=AluOp.add)
                lgn = pm.tile([P, TPC, NCLS], f32, tag="lgn")
                nc.vector.tensor_tensor(
                    out=lgn[:, 0:ntl, :],
                    in0=lg[:, 0:ntl * NCLS].rearrange("p (t c) -> p t c",
                                                      t=ntl),
                    in1=rec2[:, 0:ntl].rearrange("p (t c) -> p t c", c=1)
                        .to_broadcast([P, ntl, NCLS]),
                    op=AluOp.mult)
                nc.vector.tensor_tensor(
                    out=lgn[:, 0:ntl, :], in0=lgn[:, 0:ntl, :],
                    in1=b2_s[:].rearrange("p (t c) -> p t c", t=1)
                        .to_broadcast([P, ntl, NCLS]),
                    op=AluOp.add)
                mx = pm.tile([P, TPC, 1], f32, tag="mx")
                nc.vector.tensor_reduce(out=mx[:, 0:ntl, :],
                                        in_=lgn[:, 0:ntl, :],
                                        axis=AxX, op=AluOp.max)
                om = pm.tile([P, TPC, NCLS], f32, tag="om")
                nc.vector.tensor_tensor(
                    out=om[:, 0:ntl, :], in0=lgn[:, 0:ntl, :],
                    in1=mx[:, 0:ntl, :].to_broadcast([P, ntl, NCLS]),
                    op=AluOp.subtract)
                ex = pm.tile([P, TPC, NCLS], f32, tag="ex")
                nc.scalar.activation(ex[:, 0:ntl, :], om[:, 0:ntl, :],
                                     Act.Exp)
                s_t = pm.tile([P, TPC, 1], f32, tag="s2")
                nc.vector.tensor_reduce(out=s_t[:, 0:ntl, :],
                                        in_=ex[:, 0:ntl, :],
                                        axis=AxX, op=AluOp.add)
                ls = pm.tile([P, TPC, 1], f32, tag="ls")
                nc.scalar.activation(ls[:, 0:ntl, :], s_t[:, 0:ntl, :],
                                     Act.Ln)
                res = pm.tile([P, TPC, NCLS], f32, tag="res")
                nc.vector.tensor_tensor(
                    out=res[:, 0:ntl, :], in0=om[:, 0:ntl, :],
                    in1=ls[:, 0:ntl, :].to_broadcast([P, ntl, NCLS]),
                    op=AluOp.subtract)
                nc.sync.dma_start(
                    out=t_out[a * P:(a + ntl) * P, :].rearrange(
                        "(q p) c -> p q c", q=ntl),
                    in_=res[:, 0:ntl, :])
        es.close()

    nc.compile()
    return nc


# ----------------------------------------------------------------------------
# entry point
# ----------------------------------------------------------------------------

_CACHE = {}


def make_in_maps(pre, x, W1, att_src1, att_dst1, b1, W2, att_src2, att_dst2,
                 b2):
    node_of_local = pre["node_of_local"]
    x_bf = _to_bf16(np.vstack([np.asarray(x, np.float32),
                               np.zeros((1, F_IN), np.float32)]))
    W1n = np.asarray(W1, np.float32)
    W1r = W1n.reshape(F_IN, HEADS, HID)
    u1 = np.einsum("fhc,hc->fh", W1r, np.asarray(att_src1, np.float32))
    v1 = np.einsum("fhc,hc->fh", W1r, np.asarray(att_dst1, np.float32))
    W2n = np.asarray(W2, np.float32)
    u2 = W2n @ np.asarray(att_src2, np.float32).reshape(NCLS)
    v2 = W2n @ np.asarray(att_dst2, np.float32).reshape(NCLS)
    W2e = np.concatenate([W2n, u2[:, None], v2[:, None]], axis=1)
    bcast = lambda v, w: np.tile(np.asarray(v, np.float32).reshape(1, w), (P, 1))
    common = {
        "u1v1": _to_bf16(np.concatenate([u1, v1], axis=1)),
        "W1f": _to_bf16(np.concatenate([W1n, u1, v1], axis=1)),
        "W2e": _to_bf16(W2e),
        "b1b": bcast(b1, FW),
        "b2b": bcast(b2, NCLS),
        "ident": _to_bf16(np.eye(P, dtype=np.float32)),
    }
    in_maps = []
    for c in range(NCORES):
        xs = pre["xe_src"][c]                       # [P, SWu]
        A = x_bf[xs]                                # [P, SWu, 128]
        xeT = np.ascontiguousarray(A.transpose(2, 1, 0)).reshape(P, -1)
        xT = np.ascontiguousarray(x_bf[node_of_local[c]].T)
        in_maps.append({
            "xT": xT,
            "xeT": xeT,
            "idxA": pre["idxA"][c],
            "maskin": _to_bf16(pre["mask_all"][c]),
            **common,
        })
    return in_maps, node_of_local


def run_gat(x, edge_index, W1, att_src1, att_dst1, b1, W2, att_src2, att_dst2,
            b2, n_tiles_per_core):
    N = x.shape[0]
    pre = preprocess(N, np.asarray(edge_index, np.int64), n_tiles_per_core)

    key = (N, pre["NT"], pre["SWu"], tuple(pre["chunks"]), 1)
    if key not in _CACHE:
        _CACHE[key] = build_program(pre["NT"], pre["SWu"], pre["chunks"],
                                    pre["C_k"], n_reps=1)
    nc = _CACHE[key]

    in_maps, node_of_local = make_in_maps(pre, x, W1, att_src1, att_dst1, b1,
                                          W2, att_src2, att_dst2, b2)
    res = run_bass_kernel_spmd(nc, in_maps, core_ids=list(range(NCORES)))

    out = np.empty((N, NCLS), np.float32)
    for c in range(NCORES):
        o = res.results[c]["OUT"]
        mask = node_of_local[c] >= 0
        out[node_of_local[c][mask]] = o[mask]
    return out


def _auto_nt(n):
    NT = (n + NCORES * P - 1) // (NCORES * P)
    if (NCORES * P * NT - n) % NCORES or NCORES * P * NT == n:
        NT += 1
    return NT


def kernel(x, edge_index, W1, att_src1, att_dst1, b1, W2, att_src2, att_dst2,
           b2):
    x = np.asarray(x)
    return run_gat(x, edge_index, W1, att_src1, att_dst1, b1, W2,
                   att_src2, att_dst2, b2, n_tiles_per_core=_auto_nt(x.shape[0]))


def cached_program_and_inputs(inputs, n_reps=1):
    """For external timing harnesses: the compiled program + per-core inputs."""
    x = np.asarray(inputs["x"])
    pre = preprocess(x.shape[0], np.asarray(inputs["edge_index"], np.int64),
                     _auto_nt(x.shape[0]))
    key = (x.shape[0], pre["NT"], pre["SWu"], tuple(pre["chunks"]), n_reps)
    if key not in _CACHE:
        _CACHE[key] = build_program(pre["NT"], pre["SWu"], pre["chunks"],
                                    pre["C_k"], n_reps=n_reps)
    nc = _CACHE[key]
    in_maps, _ = make_in_maps(pre, inputs["x"], inputs["W1"],
                              inputs["att_src1"], inputs["att_dst1"],
                              inputs["b1"], inputs["W2"],
                              inputs["att_src2"], inputs["att_dst2"],
                              inputs["b2"])
    return nc, in_maps


# revision 16
# speedup vs baseline: 1.1843x; 1.1843x over previous
"""2-layer GAT (PyG GATConv semantics) on 8 Trainium2 NeuronCores via Bass/Tile.

v8 strategy — dst-node partitioning, host-staged layer-1 edge rows, and a
logit-space layer-2 table:
 - The DMA-gather path costs ~10ns per DESCRIPTOR (size-independent 256B..
   1KB, DGE-bound), so per-edge gathers are the wall.  Layer 1 avoids them
   entirely: the host stages x[src] for every edge slot (pure data layout,
   no weights) in transposed form ("xeT"), and one PE matmul per 128-slot
   column against rhs [W1 | u1 | v1] produces the per-edge xw rows AND the
   attention dots a1s/a1d in one shot (u1/v1 fold att_src1/att_dst1 into
   W1).  Aggregation weights xw rows per head (multi-head forbids
   aggregating raw x), then h = relu(agg/den + b1).
 - Per dst tile: transpose(h) -> @[W2 | u2 | v2] yields the layer-2 logit
   rows l = h@W2 plus a2s = h.u2, a2d = h.v2 (aggregating layer 2 in logit
   space is valid: single head).  T2 rows are 256B: [l bf16 x8 | a2s f32].
 - Layer 2 needs one dma_gather per edge (h is device-computed); gathers
   alternate across 4 SWDGE queues (2-queue parallelism ~1.9x, saturated
   beyond), and phase-2 math is over 8 classes, not 128 features.
 - Uniform-width chunks: tiles are degree-bucketed; every tile in a chunk
   is padded to the chunk max degree so all per-tile vector ops fuse into
   ONE chunked instruction (segment reduces become strided 4D views).
 - The phase-2 gather ring is a separate SBUF ring from the phase-1 xeT
   ring so consecutive reps overlap (phase-1 compute hides under phase-2
   gathers); DVE-only temporaries are single-buffered to pay for it.
 - Per-node attention dst terms (a1d via phase 0, a2d via phase 1) live in
   SBUF; phase 0 is 49 tiny matmuls; the T2loc AllGather is ~40us.
"""
import sys

sys.path.insert(0, "/opt/trn_rl_repo")

import numpy as np

import concourse.bass as bass
import concourse.bacc as bacc
import concourse.mybir as mybir
import concourse.tile as tile
from concourse.tile_rust import add_dep_helper
from concourse.bass_utils import run_bass_kernel_spmd

P = 128
NCORES = 8
F_IN = 128
HEADS = 4
HID = 32
NCLS = 8
NEG_SLOPE = 0.2
EPS = 1e-20
FW = HEADS * HID              # 128 : feature width (both layers)
RW2 = 128                     # T2 row width in bf16 elems (256B pitch)
WIN = 65536                   # table window rows (int16-addressable, signed)
SHARD = 8192                  # window rows per core shard (aligned)
DUMMY_POS = WIN - 1           # trailing dummy gather target (gap row)
L2S = 8                       # bf16 col of a2s f32 in T2 rows (cols 8:10)

f32 = mybir.dt.float32
bf16 = mybir.dt.bfloat16
i16 = mybir.dt.int16

CHUNK_COLS = 48               # slot-columns per chunk (uniform tile width)


def _to_bf16(a):
    import ml_dtypes
    return np.asarray(a, np.float32).astype(ml_dtypes.bfloat16)


# ----------------------------------------------------------------------------
# host-side graph preprocessing
# ----------------------------------------------------------------------------

def preprocess(N, edge_index, n_tiles_per_core):
    """Uniform-width slot layout + per-core edge staging / gather indices."""
    NT = n_tiles_per_core
    slots_pc = NT * P
    total_slots = slots_pc * NCORES
    npad_pc = (total_slots - N) // NCORES
    assert npad_pc * NCORES == total_slots - N and npad_pc >= 1
    assert slots_pc <= SHARD and total_slots <= WIN

    src = np.concatenate([edge_index[0], np.arange(N, dtype=np.int64)]).astype(np.int64)
    dst = np.concatenate([edge_index[1], np.arange(N, dtype=np.int64)]).astype(np.int64)

    deg = np.bincount(dst, minlength=N)  # in-degree incl self loop
    order = np.argsort(deg, kind="stable")  # ascending

    cap0 = (P - npad_pc) * NCORES
    r = np.arange(N)
    core = np.empty(N, np.int64)
    t_of = np.empty(N, np.int64)
    d_of = np.empty(N, np.int64)
    m0 = r < cap0
    core[m0] = r[m0] % NCORES
    t_of[m0] = 0
    d_of[m0] = npad_pc + r[m0] // NCORES
    r2 = r[~m0] - cap0
    core[~m0] = r2 % NCORES
    t_of[~m0] = 1 + r2 // (P * NCORES)
    d_of[~m0] = (r2 % (P * NCORES)) // NCORES
    assert t_of.max() < NT

    # window position of each node's table row (for the layer-2 gather)
    pos_sorted = core * SHARD + t_of * P + d_of
    pos_of = np.empty(N, np.int64)
    pos_of[order] = pos_sorted

    W_prog = np.zeros(NT, np.int64)
    np.maximum.at(W_prog, t_of, deg[order])
    W_prog = np.maximum(W_prog, 1)
    assert W_prog.max() <= CHUNK_COLS

    # uniform-width chunks: (a, b, Wc); W_prog is nondecreasing
    chunks = []
    t0 = 0
    while t0 < NT:
        b = t0 + 1
        while b < NT and (b + 1 - t0) * W_prog[b] <= CHUNK_COLS:
            b += 1
        chunks.append((t0, b, int(W_prog[b - 1])))
        t0 = b
    K = len(chunks)
    C_k = [0]
    for (a, b, Wc) in chunks:
        C_k.append(C_k[-1] + (b - a) * Wc)
    SWu = C_k[-1]
    base_col = np.zeros(NT, np.int64)
    for k, (a, b, Wc) in enumerate(chunks):
        for t in range(a, b):
            base_col[t] = C_k[k] + (t - a) * Wc

    lslot_sorted = t_of * P + d_of
    lslot = np.empty(N, np.int64)
    lslot[order] = lslot_sorted
    core_of = np.empty(N, np.int64)
    core_of[order] = core

    dkey = core_of[dst] * slots_pc + lslot[dst]
    is_self = (src == dst)
    eorder = np.argsort(dkey * 2 + (~is_self).astype(np.int64), kind="stable")
    dk_sorted = dkey[eorder]
    src_sorted = src[eorder]
    starts = np.searchsorted(dk_sorted, np.arange(total_slots))
    k_idx = np.arange(len(dk_sorted)) - starts[dk_sorted]

    e_core = dk_sorted // slots_pc
    e_rem = dk_sorted % slots_pc
    e_t = e_rem // P
    e_d = e_rem % P
    e_col = base_col[e_t] + k_idx
    assert (k_idx < W_prog[e_t]).all()

    # per-edge-slot source node id (pad slots -> node N = zero row)
    xe_src = np.full((NCORES, P, SWu), N, np.int64)
    xe_src[e_core, e_d, e_col] = src_sorted
    mask_all = np.zeros((NCORES, P, SWu), np.float32)
    mask_all[e_core, e_d, e_col] = 1.0

    # layer-2 gather positions, staged with one trailing dummy col per chunk
    posR = np.zeros((NCORES, P, SWu), np.int64)  # pad slots -> position 0
    posR[e_core, e_d, e_col] = pos_of[src_sorted]
    posA = np.full((NCORES, P, SWu + K), DUMMY_POS, np.int64)
    for k, (a, b, Wc) in enumerate(chunks):
        cc = (b - a) * Wc
        posA[:, :, C_k[k] + k:C_k[k] + k + cc] = posR[:, :, C_k[k]:C_k[k] + cc]

    idx16 = ((posA ^ 0x8000) & 0xFFFF).astype(np.uint16)
    idxA = np.zeros((NCORES, P, (SWu + K) * 8), np.int16)
    for c in range(NCORES):
        flat = idx16[c].T.ravel()
        w16 = flat.reshape(-1, 16).T
        idxA[c] = np.tile(w16, (8, 1)).view(np.int16)

    node_of_local = np.full((NCORES, slots_pc), -1, np.int64)
    node_of_local[core_of, lslot] = np.arange(N)

    return dict(
        NT=NT, SWu=SWu, chunks=chunks, C_k=C_k, npad_pc=int(npad_pc),
        xe_src=xe_src, idxA=idxA, mask_all=mask_all, pos_of=pos_of,
        node_of_local=node_of_local, slots_pc=slots_pc,
    )


# ----------------------------------------------------------------------------
# device program
# ----------------------------------------------------------------------------

def build_program(NT, SWu, chunks, C_k, n_reps=1):
    NSH = NT * P
    K = len(chunks)
    TPC = max(b - a for a, b, _ in chunks)
    CC = CHUNK_COLS
    nc = bacc.Bacc("TRN2", target_bir_lowering=False, debug=False,
                   num_devices=NCORES, num_swdge_queues=4)

    tens = {}
    tens["xT"] = nc.dram_tensor("xT", [P, NSH], bf16, kind="ExternalInput")
    tens["xeT"] = nc.dram_tensor("xeT", [P, SWu * FW], bf16,
                                 kind="ExternalInput")
    tens["idxA"] = nc.dram_tensor("idxA", [P, (SWu + K) * 8], i16,
                                  kind="ExternalInput")
    tens["maskin"] = nc.dram_tensor("maskin", [P, SWu], bf16,
                                    kind="ExternalInput")
    tens["u1v1"] = nc.dram_tensor("u1v1", [F_IN, 2 * HEADS], bf16,
                                  kind="ExternalInput")
    tens["W1f"] = nc.dram_tensor("W1f", [F_IN, FW + 2 * HEADS], bf16,
                                 kind="ExternalInput")
    tens["W2e"] = nc.dram_tensor("W2e", [FW, NCLS + 2], bf16,
                                 kind="ExternalInput")
    tens["b1b"] = nc.dram_tensor("b1b", [P, FW], f32, kind="ExternalInput")
    tens["b2b"] = nc.dram_tensor("b2b", [P, NCLS], f32, kind="ExternalInput")
    tens["ident"] = nc.dram_tensor("ident", [P, P], bf16,
                                   kind="ExternalInput")
    t_out = nc.dram_tensor("OUT", [NSH, NCLS], f32, kind="ExternalOutput")

    AluOp = mybir.AluOpType
    Act = mybir.ActivationFunctionType
    AxX = mybir.AxisListType.X

    with tile.TileContext(nc) as tc:
        from contextlib import ExitStack
        es = ExitStack()
        cp = es.enter_context(tc.tile_pool(name="const", bufs=1))
        p0 = es.enter_context(tc.tile_pool(name="p0", bufs=3))
        pg = es.enter_context(tc.tile_pool(name="pg", bufs=2))
        pg3 = es.enter_context(tc.tile_pool(name="pg3", bufs=3))
        pm = es.enter_context(tc.tile_pool(name="pm", bufs=2))
        pm1 = es.enter_context(tc.tile_pool(name="pm1", bufs=1))
        psp = es.enter_context(tc.tile_pool(name="ps", bufs=1, space="PSUM"))
        psq = es.enter_context(tc.tile_pool(name="psq", bufs=2, space="PSUM"))
        dram = es.enter_context(tc.tile_pool(name="dram", bufs=1, space="DRAM"))

        u1v1_s = cp.tile([F_IN, 2 * HEADS], bf16)
        W1f_s = cp.tile([F_IN, FW + 2 * HEADS], bf16)
        W2e_s = cp.tile([FW, NCLS + 2], bf16)
        b1_s = cp.tile([P, FW], f32)
        b2_s = cp.tile([P, NCLS], f32)
        id_s = cp.tile([P, P], bf16)
        mask_s = cp.tile([P, SWu], bf16)
        idx_s = cp.tile([P, (SWu + K) * 8], i16)
        for sb_t, name in [(u1v1_s, "u1v1"), (W1f_s, "W1f"), (W2e_s, "W2e"),
                           (b1_s, "b1b"), (b2_s, "b2b"), (id_s, "ident"),
                           (mask_s, "maskin"), (idx_s, "idxA")]:
            nc.sync.dma_start(out=sb_t[:], in_=tens[name][:])
        adT = cp.tile([P, NT, 2 * HEADS], f32)    # [a1s|a1d] per own node
        adT2 = cp.tile([P, NT], f32)              # a2d per own node

        T2loc = dram.tile([SHARD, RW2], bf16)
        T2r = [dram.tile([WIN, RW2], bf16, addr_space="Shared",
                         name=f"T2r{r}") for r in range(n_reps)]

        for rep in range(n_reps):
            # ---- phase 0: adT[:, t, :] = [x.u1 | x.v1] for own nodes
            BLK = 8
            for tb in range(0, NT, BLK):
                nb = min(BLK, NT - tb)
                xTb = p0.tile([P, BLK * P], bf16, tag="xTblk")
                nc.sync.dma_start(out=xTb[:, 0:nb * P],
                                  in_=tens["xT"][:, tb * P:(tb + nb) * P])
                ps0 = psp.tile([P, BLK, 2 * HEADS], f32, space="PSUM",
                               tag="ps0")
                for q in range(nb):
                    nc.tensor.matmul(out=ps0[:, q, :],
                                     lhsT=xTb[:, q * P:(q + 1) * P],
                                     rhs=u1v1_s[:], start=True, stop=True)
                nc.scalar.copy(adT[:, tb:tb + nb, :], ps0[:, 0:nb, :])

            # ---- phase 1
            for k in range(K):
                a, b, Wc = chunks[k]
                ntl = b - a
                cc = ntl * Wc
                c0 = C_k[k]
                xeTc = pg.tile([P, CC + 1, FW], bf16, tag="xeT")
                nc.sync.dma_start(
                    out=xeTc[:, 0:cc, :].rearrange("p c f -> p (c f)"),
                    in_=tens["xeT"][:, c0 * FW:(c0 + cc) * FW])
                # per-edge xw rows + attention dots: one matmul per
                # 128-slot column, rhs = [W1 | u1 | v1]
                xw_sb = pm.tile([P, CC, FW + 2 * HEADS], bf16, tag="xw")
                WGB = 3
                for g0 in range(0, cc, WGB):
                    ng = min(WGB, cc - g0)
                    psW = psq.tile([P, WGB, FW + 2 * HEADS], f32,
                                   space="PSUM", tag="psW")
                    for j in range(ng):
                        nc.tensor.matmul(out=psW[:, j, :],
                                         lhsT=xeTc[:, g0 + j, :],
                                         rhs=W1f_s[:], start=True, stop=True)
                    nc.scalar.copy(xw_sb[:, g0:g0 + ng, :], psW[:, 0:ng, :])
                e1 = pm.tile([P, CC, HEADS], f32, tag="e1")
                nc.vector.tensor_tensor(
                    out=e1[:, 0:cc, :].rearrange("p (t w) h -> p t w h",
                                                 w=Wc),
                    in0=xw_sb[:, 0:cc, FW:FW + HEADS].rearrange(
                        "p (t w) h -> p t w h", w=Wc),
                    in1=adT[:, a:b, HEADS:2 * HEADS]
                        .rearrange("p t (w h) -> p t w h", w=1)
                        .to_broadcast([P, ntl, Wc, HEADS]),
                    op=AluOp.add)
                f1 = pm.tile([P, CC, HEADS], f32, tag="f1")
                nc.scalar.activation(f1[:, 0:cc, :], e1[:, 0:cc, :],
                                     Act.Prelu, alpha=NEG_SLOPE)
                ee1 = pm.tile([P, CC, HEADS], bf16, tag="ee1")
                nc.scalar.activation(ee1[:, 0:cc, :], f1[:, 0:cc, :], Act.Exp)
                eem = pm.tile([P, CC, HEADS], bf16, tag="eem")
                nc.vector.tensor_tensor(
                    out=eem[:, 0:cc, :], in0=ee1[:, 0:cc, :],
                    in1=mask_s[:, c0:c0 + cc]
                        .rearrange("p (w h) -> p w h", h=1)
                        .to_broadcast([P, cc, HEADS]),
                    op=AluOp.mult)
                den = pm.tile([P, TPC * HEADS], f32, tag="den")
                nc.vector.tensor_reduce(
                    out=den[:, 0:ntl * HEADS].rearrange(
                        "p (t h) -> p t h", h=HEADS),
                    in_=eem[:, 0:cc, :].rearrange("p (t w) h -> p t h w",
                                                  w=Wc),
                    axis=AxX, op=AluOp.add)
                nc.vector.tensor_scalar_add(den[:, 0:ntl * HEADS],
                                            den[:, 0:ntl * HEADS], EPS)
                rec = pm.tile([P, TPC * HEADS], f32, tag="rec")
                nc.vector.reciprocal(rec[:, 0:ntl * HEADS],
                                     den[:, 0:ntl * HEADS])
                tm = pm1.tile([P, CC, FW], bf16, tag="tm")
                nc.vector.tensor_tensor(
                    out=tm[:, 0:cc, :].rearrange("p c (h k) -> p c h k",
                                                 h=HEADS),
                    in0=xw_sb[:, 0:cc, 0:FW].rearrange(
                        "p c (h k) -> p c h k", h=HEADS),
                    in1=eem[:, 0:cc, :].rearrange("p c (h k) -> p c h k",
                                                  k=1)
                        .to_broadcast([P, cc, HEADS, HID]),
                    op=AluOp.mult)
                agg = pm1.tile([P, TPC * FW], f32, tag="agg")
                nc.vector.tensor_reduce(
                    out=agg[:, 0:ntl * FW].rearrange(
                        "p (t f) -> p t f", f=FW),
                    in_=tm[:, 0:cc, :].rearrange("p (t w) f -> p t f w",
                                                 w=Wc),
                    axis=AxX, op=AluOp.add)
                agn = pm1.tile([P, TPC, FW], f32, tag="agn")
                nc.vector.tensor_tensor(
                    out=agn[:, 0:ntl, :].rearrange("p t (h k) -> p t h k",
                                                   h=HEADS),
                    in0=agg[:, 0:ntl * FW].rearrange(
                        "p (t h k) -> p t h k", t=ntl, h=HEADS),
                    in1=rec[:, 0:ntl * HEADS]
                        .rearrange("p (t h k) -> p t h k", t=ntl, k=1)
                        .to_broadcast([P, ntl, HEADS, HID]),
                    op=AluOp.mult)
                # h = relu(agn + b1); then per tile @[W2|u2|v2]
                nc.vector.tensor_tensor(
                    out=agn[:, 0:ntl, :], in0=agn[:, 0:ntl, :],
                    in1=b1_s[:].rearrange("p (t f) -> p t f", t=1)
                        .to_broadcast([P, ntl, FW]),
                    op=AluOp.add)
                hfb = pm.tile([P, TPC, FW], bf16, tag="hfb")
                nc.scalar.activation(hfb[:, 0:ntl, :], agn[:, 0:ntl, :],
                                     Act.Relu)
                ps2 = psq.tile([P, TPC, NCLS + 2], f32, space="PSUM",
                               tag="ps2")
                for i in range(ntl):
                    psT2 = psp.tile([P, P], bf16, space="PSUM", tag="psT")
                    nc.tensor.transpose(out=psT2[:], in_=hfb[:, i, :],
                                        identity=id_s[:])
                    hT = pm.tile([P, P], bf16, tag="hT")
                    nc.scalar.copy(hT[:], psT2[:])
                    nc.tensor.matmul(out=ps2[:, i, :], lhsT=hT[:],
                                     rhs=W2e_s[:], start=True, stop=True)
                hr2 = pm.tile([P, TPC, RW2], bf16, tag="hr2")
                nc.scalar.copy(hr2[:, 0:ntl, 0:NCLS], ps2[:, 0:ntl, 0:NCLS])
                nc.scalar.copy(hr2[:, 0:ntl, L2S:L2S + 2].bitcast(f32),
                               ps2[:, 0:ntl, NCLS:NCLS + 1])
                nc.scalar.copy(adT2[:, a:b], ps2[:, 0:ntl, NCLS + 1])
                nc.sync.dma_start(
                    out=T2loc[a * P:(a + ntl) * P, :].rearrange(
                        "(q p) f -> p q f", q=ntl),
                    in_=hr2[:, 0:ntl, :])

            T2 = T2r[rep]
            ci = nc.gpsimd.collective_compute(
                "AllGather", AluOp.bypass,
                replica_groups=[list(range(NCORES))],
                ins=[T2loc[:].opt()], outs=[T2[:].opt()])

            # ---- phase 2
            for k in range(K):
                a, b, Wc = chunks[k]
                ntl = b - a
                cc = ntl * Wc
                c0 = C_k[k]
                sc = c0 + k
                g2 = pg3.tile([P, CC + 1, FW], bf16, tag="g2")
                gi = nc.gpsimd.dma_gather(
                    g2[:, 0:cc + 1, :], T2[SHARD * 4:WIN, :],
                    idx_s[:, sc * 8:(sc + cc + 1) * 8],
                    (cc + 1) * P, (cc + 1) * P, RW2, single_packet=False,
                    queue_num=k % 4)
                add_dep_helper(gi.ins, ci.ins, True,
                               "gather reads below tracked base via neg idx")
                e2 = pm.tile([P, CC, 1], f32, tag="e2")
                nc.vector.tensor_tensor(
                    out=e2[:, 0:cc, :].rearrange("p (t w) c -> p t w c",
                                                 w=Wc),
                    in0=g2[:, 0:cc, L2S:L2S + 2].bitcast(f32)
                        .rearrange("p (t w) c -> p t w c", w=Wc),
                    in1=adT2[:, a:b].rearrange("p (t w) -> p t w", w=1)
                        .rearrange("p t (w c) -> p t w c", c=1)
                        .to_broadcast([P, ntl, Wc, 1]),
                    op=AluOp.add)
                f2 = pm.tile([P, CC, 1], f32, tag="f2")
                nc.scalar.activation(f2[:, 0:cc, :], e2[:, 0:cc, :],
                                     Act.Prelu, alpha=NEG_SLOPE)
                ee2 = pm.tile([P, CC], bf16, tag="ee2")
                nc.scalar.activation(ee2[:, 0:cc], f2[:, 0:cc, 0], Act.Exp)
                eem2 = pm.tile([P, CC], bf16, tag="eem2")
                nc.vector.tensor_tensor(out=eem2[:, 0:cc], in0=ee2[:, 0:cc],
                                        in1=mask_s[:, c0:c0 + cc],
                                        op=AluOp.mult)
                den2 = pm.tile([P, TPC], f32, tag="den2")
                nc.vector.tensor_reduce(
                    out=den2[:, 0:ntl],
                    in_=eem2[:, 0:cc].rearrange("p (t w) -> p t w", w=Wc),
                    axis=AxX, op=AluOp.add)
                nc.vector.tensor_scalar_add(den2[:, 0:ntl], den2[:, 0:ntl],
                                            EPS)
                rec2 = pm.tile([P, TPC], f32, tag="rec2")
                nc.vector.reciprocal(rec2[:, 0:ntl], den2[:, 0:ntl])
                tm2 = pm.tile([P, CC, NCLS], bf16, tag="tm2")
                nc.vector.tensor_tensor(
                    out=tm2[:, 0:cc, :], in0=g2[:, 0:cc, 0:NCLS],
                    in1=eem2[:, 0:cc].rearrange("p (c k) -> p c k", k=1)
                        .to_broadcast([P, cc, NCLS]),
                    op=AluOp.mult)
                lg = pm.tile([P, TPC * NCLS], f32, tag="lg")
                nc.vector.tensor_reduce(
                    out=lg[:, 0:ntl * NCLS].rearrange(
                        "p (t c) -> p t c", c=NCLS),
                    in_=tm2[:, 0:cc, :].rearrange("p (t w) c -> p t c w",
                                                  w=Wc),
                    axis=AxX, op=AluOp.add)
                lgn = pm.tile([P, TPC, NCLS], f32, tag="lgn")
                nc.vector.tensor_tensor(
                    out=lgn[:, 0:ntl, :],
                    in0=lg[:, 0:ntl * NCLS].rearrange("p (t c) -> p t c",
                                                      t=ntl),
                    in1=rec2[:, 0:ntl].rearrange("p (t c) -> p t c", c=1)
                        .to_broadcast([P, ntl, NCLS]),
                    op=AluOp.mult)
                nc.vector.tensor_tensor(
                    out=lgn[:, 0:ntl, :], in0=lgn[:, 0:ntl, :],
                    in1=b2_s[:].rearrange("p (t c) -> p t c", t=1)
                        .to_broadcast([P, ntl, NCLS]),
                    op=AluOp.add)
                mx = pm.tile([P, TPC, 1], f32, tag="mx")
                nc.vector.tensor_reduce(out=mx[:, 0:ntl, :],
                                        in_=lgn[:, 0:ntl, :],
                                        axis=AxX, op=AluOp.max)
                om = pm.tile([P, TPC, NCLS], f32, tag="om")
                nc.vector.tensor_tensor(
                    out=om[:, 0:ntl, :], in0=lgn[:, 0:ntl, :],
                    in1=mx[:, 0:ntl, :].to_broadcast([P, ntl, NCLS]),
                    op=AluOp.subtract)
                ex = pm.tile([P, TPC, NCLS], f32, tag="ex")
                nc.scalar.activation(ex[:, 0:ntl, :], om[:, 0:ntl, :],
                                     Act.Exp)
                s_t = pm.tile([P, TPC, 1], f32, tag="s2")
                nc.vector.tensor_reduce(out=s_t[:, 0:ntl, :],
                                        in_=ex[:, 0:ntl, :],
                                        axis=AxX, op=AluOp.add)
                ls = pm.tile([P, TPC, 1], f32, tag="ls")
                nc.scalar.activation(ls[:, 0:ntl, :], s_t[:, 0:ntl, :],
                                     Act.Ln)
                res = pm.tile([P, TPC, NCLS], f32, tag="res")
                nc.vector.tensor_tensor(
                    out=res[:, 0:ntl, :], in0=om[:, 0:ntl, :],
                    in1=ls[:, 0:ntl, :].to_broadcast([P, ntl, NCLS]),
                    op=AluOp.subtract)
                nc.sync.dma_start(
                    out=t_out[a * P:(a + ntl) * P, :].rearrange(
                        "(q p) c -> p q c", q=ntl),
                    in_=res[:, 0:ntl, :])
        es.close()

    nc.compile()
    return nc


# ----------------------------------------------------------------------------
# entry point
# ----------------------------------------------------------------------------

_CACHE = {}


def make_in_maps(pre, x, W1, att_src1, att_dst1, b1, W2, att_src2, att_dst2,
                 b2):
    node_of_local = pre["node_of_local"]
    x_bf = _to_bf16(np.vstack([np.asarray(x, np.float32),
                               np.zeros((1, F_IN), np.float32)]))
    W1n = np.asarray(W1, np.float32)
    W1r = W1n.reshape(F_IN, HEADS, HID)
    u1 = np.einsum("fhc,hc->fh", W1r, np.asarray(att_src1, np.float32))
    v1 = np.einsum("fhc,hc->fh", W1r, np.asarray(att_dst1, np.float32))
    W2n = np.asarray(W2, np.float32)
    u2 = W2n @ np.asarray(att_src2, np.float32).reshape(NCLS)
    v2 = W2n @ np.asarray(att_dst2, np.float32).reshape(NCLS)
    W2e = np.concatenate([W2n, u2[:, None], v2[:, None]], axis=1)
    bcast = lambda v, w: np.tile(np.asarray(v, np.float32).reshape(1, w), (P, 1))
    common = {
        "u1v1": _to_bf16(np.concatenate([u1, v1], axis=1)),
        "W1f": _to_bf16(np.concatenate([W1n, u1, v1], axis=1)),
        "W2e": _to_bf16(W2e),
        "b1b": bcast(b1, FW),
        "b2b": bcast(b2, NCLS),
        "ident": _to_bf16(np.eye(P, dtype=np.float32)),
    }
    in_maps = []
    for c in range(NCORES):
        xs = pre["xe_src"][c]                       # [P, SWu]
        A = x_bf[xs]                                # [P, SWu, 128]
        xeT = np.ascontiguousarray(A.transpose(2, 1, 0)).reshape(P, -1)
        xT = np.ascontiguousarray(x_bf[node_of_local[c]].T)
        in_maps.append({
            "xT": xT,
            "xeT": xeT,
            "idxA": pre["idxA"][c],
            "maskin": _to_bf16(pre["mask_all"][c]),
            **common,
        })
    return in_maps, node_of_local


def run_gat(x, edge_index, W1, att_src1, att_dst1, b1, W2, att_src2, att_dst2,
            b2, n_tiles_per_core):
    N = x.shape[0]
    pre = preprocess(N, np.asarray(edge_index, np.int64), n_tiles_per_core)

    key = (N, pre["NT"], pre["SWu"], tuple(pre["chunks"]), 1)
    if key not in _CACHE:
        _CACHE[key] = build_program(pre["NT"], pre["SWu"], pre["chunks"],
                                    pre["C_k"], n_reps=1)
    nc = _CACHE[key]

    in_maps, node_of_local = make_in_maps(pre, x, W1, att_src1, att_dst1, b1,
                                          W2, att_src2, att_dst2, b2)
    res = run_bass_kernel_spmd(nc, in_maps, core_ids=list(range(NCORES)))

    out = np.empty((N, NCLS), np.float32)
    for c in range(NCORES):
        o = res.results[c]["OUT"]
        mask = node_of_local[c] >= 0
        out[node_of_local[c][mask]] = o[mask]
    return out


def _auto_nt(n):
    NT = (n + NCORES * P - 1) // (NCORES * P)
    if (NCORES * P * NT - n) % NCORES or NCORES * P * NT == n:
        NT += 1
    return NT


def kernel(x, edge_index, W1, att_src1, att_dst1, b1, W2, att_src2, att_dst2,
           b2):
    x = np.asarray(x)
    return run_gat(x, edge_index, W1, att_src1, att_dst1, b1, W2,
                   att_src2, att_dst2, b2, n_tiles_per_core=_auto_nt(x.shape[0]))


def cached_program_and_inputs(inputs, n_reps=1):
    """For external timing harnesses: the compiled program + per-core inputs."""
    x = np.asarray(inputs["x"])
    pre = preprocess(x.shape[0], np.asarray(inputs["edge_index"], np.int64),
                     _auto_nt(x.shape[0]))
    key = (x.shape[0], pre["NT"], pre["SWu"], tuple(pre["chunks"]), n_reps)
    if key not in _CACHE:
        _CACHE[key] = build_program(pre["NT"], pre["SWu"], pre["chunks"],
                                    pre["C_k"], n_reps=n_reps)
    nc = _CACHE[key]
    in_maps, _ = make_in_maps(pre, inputs["x"], inputs["W1"],
                              inputs["att_src1"], inputs["att_dst1"],
                              inputs["b1"], inputs["W2"],
                              inputs["att_src2"], inputs["att_dst2"],
                              inputs["b2"])
    return nc, in_maps
